# revision 1
# baseline (speedup 1.0000x reference)
"""MLA (DeepSeek-style multi-head latent attention) kernel for Trainium2.

Problem: nn_MultiHeadAttention_28243704939173
  B=2, S=2048, D=2048, H=16, KV_RANK=512, NOPE=128, ROPE=64, V_HD=128.

Sharding (8 NeuronCores): DP=2 over batch x TP=4 over heads (4 heads per
core); the kv latent is computed replicated on every TP rank (as in real
MLA serving). Each core produces its heads' partial wo projection; the
host sums the 4 TP partials per batch element and adds wo_b.

Numerics: matmuls run in fp32r (fp32 with 11-bit mantissa; full PE rate)
accumulating into fp32 PSUM. Softmax skips the max-subtraction pass
(|scores * scale| < ~3 for this problem family so exp cannot overflow;
masked scores map to exp == 0 exactly). The per-(head, q-block) softmax
normalizer 1/Z folds into the PV-result copy (q on partitions there).
"""
import os
import numpy as np
from contextlib import ExitStack

import concourse.bass as bass
import concourse.bacc as bacc
import concourse.mybir as mybir
import concourse.tile as tile
from concourse import bass_utils

F32 = mybir.dt.float32
F32R = mybir.dt.float32r
AF = mybir.ActivationFunctionType
ALU = mybir.AluOpType
AX = mybir.AxisListType

B, S, D = 2, 2048, 2048
H = 16
KV = 512
NOPE, ROPE = 128, 64
QK_HD = NOPE + ROPE
V_HD = 128
SCALE = float(QK_HD) ** -0.5
EPS = 1.1920929e-07
NEG = -1.0e5  # mask addend; NEG*SCALE ~ -7220 -> exp underflows to exactly 0
HL = 4        # local heads per core (TP degree 4)
TP = 4
N_CORES = 8
KD = D // 128  # contraction chunks over the model dim


def round_f32r(a: np.ndarray) -> np.ndarray:
    """Round fp32 -> fp32r (11-bit mantissa, RNE), keeping fp32 container."""
    u = np.ascontiguousarray(a, dtype=np.float32).view(np.uint32).copy()
    lsb = (u >> np.uint32(12)) & np.uint32(1)
    u += np.uint32(0x7FF) + lsb
    u &= np.uint32(0xFFFFF000)
    return u.view(np.float32)


def build(s_len: int, q_bias: bool, kv_bias: bool, max_phase: int = 4):
    NB = s_len // 128
    NG = max(s_len // 512, 1)

    nc = bacc.Bacc("TRN2", target_bir_lowering=False, debug=False)

    xt = nc.dram_tensor("xt", [NB, 128, D], F32R, kind="ExternalInput")
    wq = nc.dram_tensor("wq", [128, KD * 768], F32R, kind="ExternalInput")
    wkv = nc.dram_tensor("wkv", [128, KD * 576], F32R, kind="ExternalInput")
    wbm = nc.dram_tensor("wbm", [128, HL * KV], F32R, kind="ExternalInput")
    wvt = nc.dram_tensor("wvt", [128, HL * 512], F32R, kind="ExternalInput")
    wot = nc.dram_tensor("wot", [128, HL * D], F32R, kind="ExternalInput")
    cosq = nc.dram_tensor("cosq", [128, NB * 64], F32, kind="ExternalInput")
    sinq = nc.dram_tensor("sinq", [128, NB * 64], F32, kind="ExternalInput")
    dmask = nc.dram_tensor("dmask", [128, 128], F32, kind="ExternalInput")
    identr = nc.dram_tensor("identr", [128, 128], F32R, kind="ExternalInput")
    identf = nc.dram_tensor("identf", [128, 128], F32, kind="ExternalInput")
    if q_bias:
        qb = nc.dram_tensor("qb", [1, 768], F32R, kind="ExternalInput")
    if kv_bias:
        kvb = nc.dram_tensor("kvb", [1, 576], F32R, kind="ExternalInput")
    out = nc.dram_tensor("out", [s_len, D], F32, kind="ExternalOutput")
    qnt_dram = nc.dram_tensor("qnt_dram", [HL, 128, s_len], F32R, kind="Internal")
    ot_dram = nc.dram_tensor("ot_dram", [128, NB, HL, 128], F32R, kind="Internal")
    kpe_bnc = nc.dram_tensor("kpe_bnc", [64, s_len], F32R, kind="Internal")

    with tile.TileContext(nc) as tc, ExitStack() as ctx:
        # ---------------- pools/tensors that live across phases -------------
        persist = ctx.enter_context(tc.tile_pool(name="persist", bufs=1))
        qpepool = ctx.enter_context(tc.tile_pool(name="qpepool", bufs=2))

        kv_sb = persist.tile([128, NB * KV], F32R, tag="kv_sb")
        kvt_sb = persist.tile([128, 4 * s_len], F32R, tag="kvt_sb")
        # kpeT duplicated on both partition halves so either 64-base qpeT
        # slice can pair with a matching-base kpeT slice in the rope matmul
        kpet_sb = persist.tile([128, s_len], F32R, tag="kpet_sb")
        identr_sb = persist.tile([128, 128], F32R, tag="identr_sb")
        identf_sb = persist.tile([128, 128], F32, tag="identf_sb")
        dmask_sb = persist.tile([128, 128], F32, tag="dmask_sb")
        cosq_sb = persist.tile([128, NB * 64], F32, tag="cosq_sb")
        sinq_sb = persist.tile([128, NB * 64], F32, tag="sinq_sb")

        nc.sync.dma_start(identr_sb[:], identr.ap()[:])
        nc.sync.dma_start(identf_sb[:], identf.ap()[:])
        nc.sync.dma_start(dmask_sb[:], dmask.ap()[:])
        nc.sync.dma_start(cosq_sb[:], cosq.ap()[:])
        nc.sync.dma_start(sinq_sb[:], sinq.ap()[:])

        qpet = [qpepool.tile([128, s_len], F32R, tag="qpepool", name=f"qpet{pp}")
                for pp in range(2)]

        # ========== Phase 1: kv latent projection + rmsnorm + k rope ==========
        with tc.tile_pool(name="p1w", bufs=1) as p1w, \
                tc.tile_pool(name="p1", bufs=3) as p1, \
                tc.tile_pool(name="p1s", bufs=2) as p1s, \
                tc.tile_pool(name="ps1a", bufs=2, space="PSUM") as ps1a, \
                tc.tile_pool(name="ps1b", bufs=2, space="PSUM") as ps1b, \
                tc.tile_pool(name="ps1t", bufs=2, space="PSUM") as ps1t:
            wkv_sb = p1w.tile([128, KD * 576], F32R, tag="wkv_sb")
            for qq in range(4):
                w4 = KD * 576 // 4
                nc.sync.dma_start(wkv_sb[:, w4 * qq:w4 * (qq + 1)],
                                  wkv.ap()[:, w4 * qq:w4 * (qq + 1)])
            if kv_bias:
                kvb_sb = p1w.tile([1, 576], F32R, tag="kvb_sb")
                nc.sync.dma_start(kvb_sb[:], kvb.ap()[:])
                ones1 = p1w.tile([1, 128], F32R, tag="ones1")
                nc.vector.memset(ones1[:], 1.0)
            for s in range(NB):
                xts = p1.tile([128, D], F32R, tag="xts")
                nc.sync.dma_start(xts[:], xt.ap()[s])
                pkv = ps1a.tile([128, 512], F32, tag="pkv")
                pkp = ps1b.tile([128, 64], F32, tag="pkp")
                for k in range(KD):
                    lhs = xts[:, 128 * k:128 * (k + 1)]
                    nc.tensor.matmul(pkv[:], lhs, wkv_sb[:, 576 * k:576 * k + 512],
                                     start=(k == 0),
                                     stop=(k == KD - 1 and not kv_bias))
                    nc.tensor.matmul(pkp[:], lhs,
                                     wkv_sb[:, 576 * k + 512:576 * (k + 1)],
                                     start=(k == 0),
                                     stop=(k == KD - 1 and not kv_bias))
                if kv_bias:
                    nc.tensor.matmul(pkv[:], ones1[:], kvb_sb[:, 0:512],
                                     start=False, stop=True)
                    nc.tensor.matmul(pkp[:], ones1[:], kvb_sb[:, 512:576],
                                     start=False, stop=True)
                # rmsnorm over the 512 latent channels
                kvtile = p1.tile([128, 512], F32, tag="kvtile")
                nc.vector.tensor_copy(kvtile[:], pkv[:])
                sq = p1.tile([128, 512], F32, tag="sq")
                msq = p1s.tile([128, 1], F32, tag="msq")
                nc.scalar.activation(sq[:], kvtile[:], AF.Square, bias=0.0,
                                     scale=1.0, accum_out=msq[:])
                ms2 = p1s.tile([128, 1], F32, tag="ms2")
                nc.vector.tensor_scalar(ms2[:], msq[:], 1.0 / KV, EPS, ALU.mult,
                                        ALU.add)
                srt = p1s.tile([128, 1], F32, tag="srt")
                nc.scalar.sqrt(srt[:], ms2[:])
                rrt = p1s.tile([128, 1], F32, tag="rrt")
                nc.vector.reciprocal(rrt[:], srt[:])
                nc.vector.tensor_scalar(kv_sb[:, KV * s:KV * (s + 1)], kvtile[:],
                                        rrt[:], None, ALU.mult)
                # k_pe rope (free-dim interleaved pairs)
                kpe = p1s.tile([128, 64], F32, tag="kpe")
                nc.vector.tensor_copy(kpe[:], pkp[:])
                ksw = p1s.tile([128, 64], F32, tag="ksw")
                k3 = kpe[:].rearrange("p (i two) -> p i two", two=2)
                w3 = ksw[:].rearrange("p (i two) -> p i two", two=2)
                nc.vector.tensor_copy(w3[:, :, 0:1], k3[:, :, 1:2])
                nc.vector.tensor_copy(w3[:, :, 1:2], k3[:, :, 0:1])
                krot = p1s.tile([128, 64], F32, tag="krot")
                nc.vector.tensor_mul(krot[:], kpe[:], cosq_sb[:, 64 * s:64 * (s + 1)])
                nc.vector.tensor_mul(ksw[:], ksw[:], sinq_sb[:, 64 * s:64 * (s + 1)])
                nc.vector.tensor_add(krot[:], krot[:], ksw[:])
                ptk = ps1t.tile([64, 128], F32, tag="ptk")
                nc.tensor.transpose(ptk[:], krot[:], identf_sb[:])
                nc.vector.tensor_copy(kpet_sb[0:64, 128 * s:128 * (s + 1)], ptk[:])
                # transpose normed kv block into kvT
                for cc in range(4):
                    ptc = ps1t.tile([128, 128], F32R, tag="ptc")
                    nc.tensor.transpose(
                        ptc[:], kv_sb[:, KV * s + 128 * cc:KV * s + 128 * (cc + 1)],
                        identr_sb[:])
                    nc.vector.tensor_copy(
                        kvt_sb[:, s_len * cc + 128 * s:s_len * cc + 128 * (s + 1)],
                        ptc[:])
            # duplicate kpeT into the upper partition half via a DRAM bounce
            # (a same-tensor SBUF->SBUF DMA deadlocks on HW)
            nc.sync.dma_start(kpe_bnc.ap()[:], kpet_sb[0:64, :])
            nc.sync.dma_start(kpet_sb[64:128, :], kpe_bnc.ap()[:])

        # ========== Phase 2: q projection + q rope + transposes ==========
        if max_phase >= 2:
          with tc.tile_pool(name="p2w", bufs=1) as p2w, \
                  tc.tile_pool(name="p2", bufs=3) as p2, \
                  tc.tile_pool(name="ps2", bufs=3, space="PSUM") as ps2, \
                  tc.tile_pool(name="ps2t", bufs=2, space="PSUM") as ps2t:
            wq_sb = p2w.tile([128, KD * 768], F32R, tag="wq_sb")
            for qq in range(4):
                w4 = KD * 768 // 4
                nc.sync.dma_start(wq_sb[:, w4 * qq:w4 * (qq + 1)],
                                  wq.ap()[:, w4 * qq:w4 * (qq + 1)])
            if q_bias:
                qb_sb = p2w.tile([1, 768], F32R, tag="qb_sb")
                nc.sync.dma_start(qb_sb[:], qb.ap()[:])
                ones2 = p2w.tile([1, 128], F32R, tag="ones2")
                nc.vector.memset(ones2[:], 1.0)
            for s in range(NB):
                xts = p2.tile([128, D], F32R, tag="xts2")
                nc.sync.dma_start(xts[:], xt.ap()[s])
                pq = ps2.tile([128, 768], F32, tag="pq")
                for k in range(KD):
                    lhs = xts[:, 128 * k:128 * (k + 1)]
                    nc.tensor.matmul(pq[:, 0:512], lhs,
                                     wq_sb[:, 768 * k:768 * k + 512],
                                     start=(k == 0),
                                     stop=(k == KD - 1 and not q_bias))
                    nc.tensor.matmul(pq[:, 512:768], lhs,
                                     wq_sb[:, 768 * k + 512:768 * (k + 1)],
                                     start=(k == 0),
                                     stop=(k == KD - 1 and not q_bias))
                if q_bias:
                    nc.tensor.matmul(pq[:, 0:512], ones2[:], qb_sb[:, 0:512],
                                     start=False, stop=True)
                    nc.tensor.matmul(pq[:, 512:768], ones2[:], qb_sb[:, 512:768],
                                     start=False, stop=True)
                qsb = p2.tile([128, 768], F32, tag="qsb")
                nc.scalar.copy(qsb[:], pq[:])
                # rope on cols 512:768 (4 heads x 64 interleaved pairs)
                qsw = p2.tile([128, 256], F32, tag="qsw")
                a3 = qsb[:, 512:768].rearrange("p (i two) -> p i two", two=2)
                w3 = qsw[:].rearrange("p (i two) -> p i two", two=2)
                nc.vector.tensor_copy(w3[:, :, 0:1], a3[:, :, 1:2])
                nc.vector.tensor_copy(w3[:, :, 1:2], a3[:, :, 0:1])
                for hh in range(HL):
                    rsl = qsb[:, 512 + 64 * hh:512 + 64 * (hh + 1)]
                    ssl = qsw[:, 64 * hh:64 * (hh + 1)]
                    nc.vector.tensor_mul(rsl, rsl, cosq_sb[:, 64 * s:64 * (s + 1)])
                    nc.vector.tensor_mul(ssl, ssl, sinq_sb[:, 64 * s:64 * (s + 1)])
                    nc.vector.tensor_add(rsl, rsl, ssl)
                # transposes into qnT (via DRAM) and qpeT pair tensors
                for hh in range(HL):
                    pt2 = ps2t.tile([128, 128], F32, tag="pt2")
                    nc.tensor.transpose(pt2[:], qsb[:, 128 * hh:128 * (hh + 1)],
                                        identf_sb[:])
                    qnstg = p2.tile([128, 128], F32R, tag="qnstg")
                    nc.vector.tensor_copy(qnstg[:], pt2[:])
                    nc.sync.dma_start(qnt_dram.ap()[hh, :, 128 * s:128 * (s + 1)],
                                      qnstg[:])
                for pp in range(2):
                    pt2 = ps2t.tile([128, 128], F32, tag="pt2")
                    nc.tensor.transpose(pt2[:],
                                        qsb[:, 512 + 128 * pp:512 + 128 * (pp + 1)],
                                        identf_sb[:])
                    nc.vector.tensor_copy(qpet[pp][:, 128 * s:128 * (s + 1)], pt2[:])

        # ========== Phase 3: attention per local head ==========
        if max_phase >= 3:
          with tc.tile_pool(name="p3w", bufs=1) as p3w, \
                  tc.tile_pool(name="qatp", bufs=1) as qatp, \
                  tc.tile_pool(name="qntp", bufs=2) as qntp, \
                  tc.tile_pool(name="expp", bufs=6) as expp, \
                  tc.tile_pool(name="p3", bufs=3) as p3, \
                  tc.tile_pool(name="otstp", bufs=2) as otstp, \
                  tc.tile_pool(name="ps3s", bufs=3, space="PSUM") as ps3s, \
                  tc.tile_pool(name="ps3a", bufs=3, space="PSUM") as ps3a, \
                  tc.tile_pool(name="ps3t", bufs=2, space="PSUM") as ps3t:
            wb_sb = p3w.tile([128, HL * KV], F32R, tag="wb_sb")
            nc.sync.dma_start(wb_sb[:], wbm.ap()[:])
            wvt_sb = p3w.tile([128, HL * 512], F32R, tag="wvt_sb")
            nc.sync.dma_start(wvt_sb[:], wvt.ap()[:])
            for h in range(HL):
                # ---- absorb: qaT_h[c, q] = (qn_h @ Wb'_h)^T, cc-major ----
                qnts = qntp.tile([128, s_len], F32R, tag="qnts")
                nc.sync.dma_start(qnts[:], qnt_dram.ap()[h])
                qat = qatp.tile([128, 4 * s_len], F32R, tag="qat")
                gw0 = min(512, s_len)
                for cc in range(4):
                    for g in range(NG):
                        pa = ps3a.tile([128, 512], F32, tag="pacc")
                        nc.tensor.matmul(
                            pa[:, 0:gw0],
                            wb_sb[:, KV * h + 128 * cc:KV * h + 128 * (cc + 1)],
                            qnts[:, 512 * g:512 * g + gw0],
                            start=True, stop=True)
                        nc.scalar.copy(
                            qat[:, s_len * cc + 512 * g:s_len * cc + 512 * g + gw0],
                            pa[:, 0:gw0])
                otst = None
                for i in range(NB):
                    nk = 128 * (i + 1)
                    nts = (nk + 511) // 512
                    if i % 4 == 0:
                        otst = otstp.tile([128, 2048], F32R, tag="otst")
                    # ---- scores for q-block i over all key slices ----
                    expsl_tiles = []
                    zp = p3.tile([128, 4], F32, tag="zp")
                    for ts in range(nts):
                        t0 = 512 * ts
                        tw = min(512, nk - t0)
                        pss = ps3s.tile([128, 512], F32, tag="pss")
                        for cc in range(4):
                            nc.tensor.matmul(
                                pss[:, 0:tw],
                                qat[:, s_len * cc + 128 * i:
                                    s_len * cc + 128 * (i + 1)],
                                kvt_sb[:, s_len * cc + t0:s_len * cc + t0 + tw],
                                start=(cc == 0), stop=False, skip_group_check=True)
                        nc.tensor.matmul(
                            pss[:, 0:tw],
                            qpet[h // 2][64 * (h % 2):64 * (h % 2) + 64,
                                         128 * i:128 * (i + 1)],
                            kpet_sb[64 * (h % 2):64 * (h % 2) + 64, t0:t0 + tw],
                            start=False, stop=True, skip_group_check=True)
                        if t0 + tw == nk:
                            nc.vector.tensor_add(pss[:, tw - 128:tw],
                                                 pss[:, tw - 128:tw], dmask_sb[:])
                        expsl = expp.tile([128, 512], F32R, tag="expsl")
                        nc.scalar.activation(expsl[:, 0:tw], pss[:, 0:tw], AF.Exp,
                                             bias=0.0, scale=SCALE,
                                             accum_out=zp[:, ts:ts + 1])
                        expsl_tiles.append(expsl)
                    # ---- 1/Z for this (head, q-block) ----
                    if nts > 1:
                        zs = p3.tile([128, 1], F32, tag="zs")
                        nc.vector.reduce_sum(zs[:], zp[:, 0:nts], axis=AX.X)
                    else:
                        zs = zp
                    rq = p3.tile([128, 1], F32, tag="rq")
                    nc.vector.reciprocal(rq[:], zs[:, 0:1])
                    # ---- PV: transpose P in 4-block groups, accumulate over t ----
                    po = ps3a.tile([128, 512], F32, tag="pacc")
                    for jg in range(nts):
                        jn = min(4, (i + 1) - 4 * jg)
                        pt3 = ps3t.tile([128, 512], F32R, tag="pt3")
                        for jj in range(jn):
                            j = 4 * jg + jj
                            ts_j, off = divmod(128 * j, 512)
                            nc.tensor.transpose(pt3[:, 128 * jj:128 * (jj + 1)],
                                                expsl_tiles[ts_j][:, off:off + 128],
                                                identr_sb[:])
                        ptile = p3.tile([128, 512], F32R, tag="ptile", bufs=3)
                        nc.vector.tensor_copy(ptile[:, 0:128 * jn], pt3[:, 0:128 * jn])
                        for jj in range(jn):
                            j = 4 * jg + jj
                            nc.tensor.matmul(po[:],
                                             ptile[:, 128 * jj:128 * (jj + 1)],
                                             kv_sb[:, KV * j:KV * (j + 1)],
                                             start=(j == 0), stop=(j == i),
                                             skip_group_check=True)
                    # ---- normalize by 1/Z on the PSUM->SBUF copy ----
                    ocp = p3.tile([128, 512], F32R, tag="ocp", bufs=2)
                    nc.scalar.mul(ocp[:], po[:], rq[:])
                    # ---- transpose normalized PV into group staging ----
                    pt4 = ps3t.tile([128, 512], F32R, tag="pt3")
                    for cc in range(4):
                        nc.tensor.transpose(pt4[:, 128 * cc:128 * (cc + 1)],
                                            ocp[:, 128 * cc:128 * (cc + 1)],
                                            identr_sb[:])
                    ot4 = otst[:].rearrange("p (cc q) -> p cc q", cc=4)
                    nc.vector.tensor_copy(
                        ot4[:, :, 128 * (i % 4):128 * (i % 4 + 1)],
                        pt4[:].rearrange("p (cc q) -> p cc q", cc=4))
                    # ---- after each 4-block group: oT_h[d, q] over c-chunks ----
                    if i % 4 == 3 or i == NB - 1:
                        g = i // 4
                        gw = 128 * (i % 4 + 1)
                        pot = ps3a.tile([128, 512], F32, tag="pacc")
                        for cc in range(4):
                            nc.tensor.matmul(
                                pot[:, 0:gw],
                                wvt_sb[:, 512 * h + 128 * cc:
                                       512 * h + 128 * (cc + 1)],
                                otst[:, 512 * cc:512 * cc + gw],
                                start=(cc == 0), stop=(cc == 3))
                        otg = p3.tile([128, 512], F32R, tag="otg", bufs=2)
                        nc.vector.tensor_copy(otg[:, 0:gw], pot[:, 0:gw])
                        nc.sync.dma_start(
                            ot_dram.ap()[:, 4 * g:4 * g + gw // 128, h, :],
                            otg[:, 0:gw].rearrange("p (i c) -> p i c", c=128))

        # ========== Phase 4: wo projection ==========
        if max_phase >= 4:
          with tc.tile_pool(name="p4w", bufs=1) as p4w, \
                  tc.tile_pool(name="p4", bufs=3) as p4, \
                  tc.tile_pool(name="ps4", bufs=2, space="PSUM") as ps4:
            wot_sb = p4w.tile([128, HL * D], F32R, tag="wot_sb")
            for qq in range(4):
                w4 = HL * D // 4
                nc.sync.dma_start(wot_sb[:, w4 * qq:w4 * (qq + 1)],
                                  wot.ap()[:, w4 * qq:w4 * (qq + 1)])
            for i in range(NB):
                otq = p4.tile([128, 512], F32R, tag="otq", bufs=4)
                nc.sync.dma_start(otq[:], ot_dram.ap()[:, i])
                for n in range(D // 512):
                    pw = ps4.tile([128, 512], F32, tag="pw")
                    for dc in range(HL):
                        nc.tensor.matmul(
                            pw[:], otq[:, 128 * dc:128 * (dc + 1)],
                            wot_sb[:, D * dc + 512 * n:D * dc + 512 * (n + 1)],
                            start=(dc == 0), stop=(dc == HL - 1))
                    osb = p4.tile([128, 512], F32, tag="osb")
                    nc.scalar.copy(osb[:], pw[:])
                    nc.sync.dma_start(
                        out.ap()[128 * i:128 * (i + 1), 512 * n:512 * (n + 1)],
                        osb[:])

    nc.compile()
    return nc


def make_core_inputs(core, x, freqs, wq_w, wq_b, wkv_a_w, wkv_a_b, kv_norm_w,
                     wkv_b_w, wo_w, s_len):
    """Host-side shard + layout prep for one core."""
    b, g = core // TP, core % TP
    NB = s_len // 128
    heads = [TP * g + hh for hh in range(HL)]  # heads for TP rank g

    ins = {}
    # xt[s, p, 128k+c] = x[b, 128s+c, 128k+p]
    xb = np.ascontiguousarray(x[b, :s_len])                       # [S, D]
    xt = xb.reshape(NB, 128, KD, 128).transpose(0, 3, 2, 1)       # [s, p, k, c]
    ins["xt"] = round_f32r(np.ascontiguousarray(xt).reshape(NB, 128, D))

    # wq rows: 4x nope(128) then 4x rope(64) for local heads -> [768, D]
    wq3 = wq_w.reshape(H, QK_HD, D)
    rows = [wq3[hg, :NOPE] for hg in heads] + [wq3[hg, NOPE:] for hg in heads]
    wq_sel = np.concatenate(rows, axis=0)                         # [768, D]
    wqt = wq_sel.T.reshape(KD, 128, 768).transpose(1, 0, 2)       # [p, k, 768]
    ins["wq"] = round_f32r(np.ascontiguousarray(wqt).reshape(128, KD * 768))

    wkvt = wkv_a_w.T.reshape(KD, 128, 576).transpose(1, 0, 2)
    ins["wkv"] = round_f32r(np.ascontiguousarray(wkvt).reshape(128, KD * 576))

    wkv_b3 = wkv_b_w.reshape(H, NOPE + V_HD, KV)
    wb_cols = [wkv_b3[hg, :NOPE] * kv_norm_w[None, :] for hg in heads]
    ins["wbm"] = round_f32r(np.concatenate(wb_cols, axis=1))      # [128, 4*512]

    wvt_cols = []
    for hg in heads:
        wv = wkv_b3[hg, NOPE:] * kv_norm_w[None, :]               # [128(d), 512(c)]
        wvt_cols.append(wv.T.reshape(4, 128, 128).transpose(1, 0, 2).reshape(128, 512))
    ins["wvt"] = round_f32r(np.concatenate(wvt_cols, axis=1))     # [128, 4*512]

    wo_cols = np.concatenate([wo_w[:, hg * V_HD:(hg + 1) * V_HD] for hg in heads],
                             axis=1)                              # [D, 512]
    wot = wo_cols.T.reshape(HL, 128, D).transpose(1, 0, 2)        # [p, dc, D]
    ins["wot"] = round_f32r(np.ascontiguousarray(wot).reshape(128, HL * D))

    # rope tables in [s-block(128), 64] free-pair layout
    fr = freqs[:s_len]                                            # [S, 32]
    cos2 = np.repeat(np.cos(fr), 2, axis=1).astype(np.float32)    # [S, 64]
    sin1 = np.sin(fr)
    sin2 = np.empty((s_len, ROPE), np.float32)
    sin2[:, 0::2] = -sin1
    sin2[:, 1::2] = sin1
    ins["cosq"] = np.ascontiguousarray(
        cos2.reshape(NB, 128, 64).transpose(1, 0, 2).reshape(128, NB * 64))
    ins["sinq"] = np.ascontiguousarray(
        sin2.reshape(NB, 128, 64).transpose(1, 0, 2).reshape(128, NB * 64))

    ins["dmask"] = np.where(np.triu(np.ones((128, 128), bool), k=1),
                            np.float32(NEG), np.float32(0.0))
    eye = np.eye(128, dtype=np.float32)
    ins["identr"] = eye
    ins["identf"] = eye

    if np.any(wq_b != 0.0):
        rows_b = [wq_b.reshape(H, QK_HD)[hg, :NOPE] for hg in heads] + \
                 [wq_b.reshape(H, QK_HD)[hg, NOPE:] for hg in heads]
        ins["qb"] = round_f32r(np.concatenate(rows_b)[None, :])
    if np.any(wkv_a_b != 0.0):
        ins["kvb"] = round_f32r(wkv_a_b[None, :])
    return ins


_nc_cache = {}


def get_nc(s_len, q_bias, kv_bias):
    key = (s_len, q_bias, kv_bias)
    if key not in _nc_cache:
        _nc_cache[key] = build(s_len, q_bias, kv_bias)
    return _nc_cache[key]


def run_cores(inputs, s_len=S, trace=False):
    """Build per-core shards, run the SPMD kernel, return (out, results)."""
    x = np.asarray(inputs["x"], np.float32)
    freqs = np.asarray(inputs["freqs"], np.float32)
    wq_w = np.asarray(inputs["wq_w"], np.float32)
    wq_b = np.asarray(inputs["wq_b"], np.float32)
    wkv_a_w = np.asarray(inputs["wkv_a_w"], np.float32)
    wkv_a_b = np.asarray(inputs["wkv_a_b"], np.float32)
    kv_norm_w = np.asarray(inputs["kv_norm_w"], np.float32)
    wkv_b_w = np.asarray(inputs["wkv_b_w"], np.float32)
    wo_w = np.asarray(inputs["wo_w"], np.float32)
    wo_b = np.asarray(inputs["wo_b"], np.float32)

    q_bias = bool(np.any(wq_b != 0.0))
    kv_bias = bool(np.any(wkv_a_b != 0.0))
    nc = get_nc(s_len, q_bias, kv_bias)
    in_maps = [
        make_core_inputs(c, x, freqs, wq_w, wq_b, wkv_a_w, wkv_a_b, kv_norm_w,
                         wkv_b_w, wo_w, s_len)
        for c in range(N_CORES)
    ]
    res = bass_utils.run_bass_kernel_spmd(nc, in_maps, core_ids=list(range(N_CORES)),
                                          trace=trace)
    out = np.empty((B, s_len, D), np.float32)
    for b in range(B):
        p = [res.results[TP * b + g]["out"] for g in range(TP)]
        out[b] = (p[0] + p[1]) + (p[2] + p[3])
    out += wo_b[None, None, :]
    return out, res


def kernel(**inputs) -> np.ndarray:
    out, _ = run_cores(inputs, s_len=S, trace=False)
    return out



# revision 9
# speedup vs baseline: 1.6899x; 1.6899x over previous
"""MLA (DeepSeek-style multi-head latent attention) kernel for Trainium2, v2.

Problem: nn_MultiHeadAttention_28243704939173
  B=2, S=2048, D=2048, H=16, KV_RANK=512, NOPE=128, ROPE=64, V_HD=128.

Sharding (8 NeuronCores): DP=2 over batch x TP=4 over heads (4 heads per
core); the kv latent is computed replicated on every TP rank. Each core
produces its heads' partial wo projection (transposed, [D, S]); the host
sums the 4 TP partials per batch element and adds wo_b.

v2 structure (vs v1):
  * Per-head K/V materialization: k_eff_h = wb_h @ kv_latent^T (128-d) and
    v_eff_h = kv_latent @ wv_h^T (128-d) are computed once (cheap GEMMs),
    so scores contract over 192 dims and PV over per-128 blocks instead of
    doing both in the 512-d latent space. ~2.4x less PE work in attention.
  * Attention path entirely in bf16 (inputs quantized ~0.1%; final rel err
    ~1e-3 vs the 2e-2 gate). x-projections stay fp32r.
  * Single fused pass over x computes kv latent + q (nope/rope) + k-rope
    in one GEMM sweep (x loaded once).
  * All weights prefetched on the Activation DMA queue while x streams on
    the SP queue; no DRAM bounces for intermediates.
  * Softmax normalization folded into P tiles in-place (gpsimd) before the
    PE transposes; per-(head, q-group) work software-pipelined so the PE
    never waits on the softmax chain.
"""
import numpy as np
from contextlib import ExitStack

import ml_dtypes

import concourse.bass as bass
import concourse.bacc as bacc
import concourse.mybir as mybir
import concourse.tile as tile
from concourse import bass_utils

F32 = mybir.dt.float32
F32R = mybir.dt.float32r
BF16 = mybir.dt.bfloat16
AF = mybir.ActivationFunctionType
ALU = mybir.AluOpType
AX = mybir.AxisListType

B, S, D = 2, 2048, 2048
H = 16
KV = 512
NOPE, ROPE = 128, 64
QK_HD = NOPE + ROPE
V_HD = 128
SCALE = float(QK_HD) ** -0.5
EPS = 1.1920929e-07
NEG = -1.0e5  # mask addend; NEG*SCALE ~ -7220 -> exp underflows to exactly 0
HL = 4        # local heads per core (TP degree 4)
TP = 4
N_CORES = 8
KD = D // 128   # contraction chunks over the model dim
QW = 832        # fused q-projection width: 4*128 nope + 4*64 qrope + 64 krope


def round_f32r(a: np.ndarray) -> np.ndarray:
    """Round fp32 -> fp32r (11-bit mantissa, RNE), keeping fp32 container."""
    u = np.ascontiguousarray(a, dtype=np.float32).view(np.uint32).copy()
    lsb = (u >> np.uint32(12)) & np.uint32(1)
    u += np.uint32(0x7FF) + lsb
    u &= np.uint32(0xFFFFF000)
    return u.view(np.float32)


def to_bf16(a: np.ndarray) -> np.ndarray:
    return np.ascontiguousarray(a).astype(ml_dtypes.bfloat16)


def build(s_len: int, q_bias: bool, kv_bias: bool):
    NB = s_len // 128

    nc = bacc.Bacc("TRN2", target_bir_lowering=False, debug=False)

    xt = nc.dram_tensor("xt", [NB, 128, D], F32R, kind="ExternalInput")
    wq = nc.dram_tensor("wq", [128, KD * QW], F32R, kind="ExternalInput")
    wkv = nc.dram_tensor("wkv", [128, KD * 512], F32R, kind="ExternalInput")
    wbmt = nc.dram_tensor("wbmt", [128, HL * 512], BF16, kind="ExternalInput")
    wvt = nc.dram_tensor("wvt", [128, HL * 512], BF16, kind="ExternalInput")
    wot = nc.dram_tensor("wot", [128, HL * D], BF16, kind="ExternalInput")
    cosq = nc.dram_tensor("cosq", [128, NB * 64], F32, kind="ExternalInput")
    sinq = nc.dram_tensor("sinq", [128, NB * 64], F32, kind="ExternalInput")
    dmask = nc.dram_tensor("dmask", [128, 128], F32, kind="ExternalInput")
    identb = nc.dram_tensor("identb", [128, 128], BF16, kind="ExternalInput")
    if q_bias:
        qb = nc.dram_tensor("qb", [1, QW], F32R, kind="ExternalInput")
    if kv_bias:
        kvb = nc.dram_tensor("kvb", [1, 512], F32R, kind="ExternalInput")
    outT = nc.dram_tensor("outT", [D // 128, 128, s_len], F32,
                          kind="ExternalOutput")

    with tile.TileContext(nc) as tc, ExitStack() as ctx:
        # ---------------- persistent tensors (whole kernel) -----------------
        pe = ctx.enter_context(tc.tile_pool(name="pe", bufs=1))
        identb_sb = pe.tile([128, 128], BF16, tag="identb_sb")
        dmask_sb = pe.tile([128, 128], F32, tag="dmask_sb")
        qnt = pe.tile([128, HL * s_len], BF16, tag="qnt")       # [d, (h,q)]
        qpet = [pe.tile([128, s_len], BF16, tag=f"qpet{pp}", name=f"qpet{pp}")
                for pp in range(2)]                             # [(2h,r), q]
        kpet = pe.tile([128, s_len], BF16, tag="kpet")          # [r x2, t]
        kvt = pe.tile([128, 4 * s_len], BF16, tag="kvt")        # [c, (cc,t)]
        wbmt_sb = pe.tile([128, HL * 512], BF16, tag="wbmt_sb")
        wvt_sb = pe.tile([128, HL * 512], BF16, tag="wvt_sb")

        nc.sync.dma_start(identb_sb[:], identb.ap()[:])
        nc.sync.dma_start(dmask_sb[:], dmask.ap()[:])

        # ========== Phase 12: fused kv-latent + q/k-rope projections ========
        with tc.tile_pool(name="p12w", bufs=1) as p12w, \
                tc.tile_pool(name="p12", bufs=3) as p12, \
                tc.tile_pool(name="p12s", bufs=4) as p12s, \
                tc.tile_pool(name="pskv", bufs=2, space="PSUM") as pskv, \
                tc.tile_pool(name="psq1", bufs=2, space="PSUM") as psq1, \
                tc.tile_pool(name="psq2", bufs=2, space="PSUM") as psq2, \
                tc.tile_pool(name="pst", bufs=2, space="PSUM") as pst:
            # x block 0 first on the SP queue so compute can start early
            xtc0 = [p12.tile([128, 512], F32R, tag=f"xtc{q}", name=f"xtc{q}_0")
                    for q in range(4)]
            for q in range(4):
                nc.sync.dma_start(xtc0[q][:], xt.ap()[0][:, 512 * q:512 * (q + 1)])
            # weights stream on the Activation HWDGE queue in parallel
            wkvc = []
            for k in range(KD):
                wt = p12w.tile([128, 512], F32R, tag="wkvc", name=f"wkvc{k}",
                               bufs=KD)
                nc.scalar.dma_start(wt[:], wkv.ap()[:, 512 * k:512 * (k + 1)])
                wkvc.append(wt)
            wqc = []
            for k in range(KD):
                wt = p12w.tile([128, QW], F32R, tag="wqc", name=f"wqc{k}",
                               bufs=KD)
                nc.scalar.dma_start(wt[:], wq.ap()[:, QW * k:QW * (k + 1)])
                wqc.append(wt)
            cosq_sb = p12w.tile([128, NB * 64], F32, tag="cosq_sb")
            sinq_sb = p12w.tile([128, NB * 64], F32, tag="sinq_sb")
            nc.sync.dma_start(cosq_sb[:], cosq.ap()[:])
            nc.sync.dma_start(sinq_sb[:], sinq.ap()[:])
            nc.scalar.dma_start(wbmt_sb[:], wbmt.ap()[:])
            nc.scalar.dma_start(wvt_sb[:], wvt.ap()[:])
            if q_bias or kv_bias:
                ones1 = p12w.tile([1, 128], F32R, tag="ones1")
                nc.vector.memset(ones1[:], 1.0)
            if q_bias:
                qb_sb = p12w.tile([1, QW], F32R, tag="qb_sb")
                nc.scalar.dma_start(qb_sb[:], qb.ap()[:])
            if kv_bias:
                kvb_sb = p12w.tile([1, 512], F32R, tag="kvb_sb")
                nc.scalar.dma_start(kvb_sb[:], kvb.ap()[:])

            qnt4 = qnt[:].rearrange("p (h n c) -> p h n c", h=HL, n=NB)
            kvt4 = kvt[:].rearrange("p (cc n c) -> p cc n c", cc=4, n=NB)

            def emit_tr(st):
                """PE transposes for block st (lagged one iteration)."""
                s, kv_bf, qn_bf, qro_bf = st
                ptc = pst.tile([128, 512], BF16, tag="ptc")
                for cc in range(4):
                    nc.tensor.transpose(ptc[:, 128 * cc:128 * (cc + 1)],
                                        kv_bf[:, 128 * cc:128 * (cc + 1)],
                                        identb_sb[:])
                nc.vector.tensor_copy(
                    kvt4[:, :, s, :],
                    ptc[:].rearrange("p (cc c) -> p cc c", cc=4))
                ptq = pst.tile([128, 512], BF16, tag="ptc")
                for hh in range(HL):
                    nc.tensor.transpose(ptq[:, 128 * hh:128 * (hh + 1)],
                                        qn_bf[:, 128 * hh:128 * (hh + 1)],
                                        identb_sb[:])
                nc.vector.tensor_copy(
                    qnt4[:, :, s, :],
                    ptq[:].rearrange("p (h c) -> p h c", h=HL))
                ptp = pst.tile([128, 512], BF16, tag="ptc")
                for pp in range(2):
                    nc.tensor.transpose(ptp[:, 128 * pp:128 * (pp + 1)],
                                        qro_bf[:, 128 * pp:128 * (pp + 1)],
                                        identb_sb[:])
                    nc.vector.tensor_copy(qpet[pp][:, 128 * s:128 * (s + 1)],
                                          ptp[:, 128 * pp:128 * (pp + 1)])
                # kpeT duplicated on both partition halves (the rope matmul
                # pairs it with either half of qpet); DVE cannot cross
                # partitions, so transpose twice with an explicit placement.
                nc.tensor.transpose(ptp[0:64, 256:384], qro_bf[:, 256:320],
                                    identb_sb[:], tile_position=(0, 0))
                nc.tensor.transpose(ptp[64:128, 256:384], qro_bf[:, 256:320],
                                    identb_sb[:], tile_position=(0, 64))
                nc.vector.tensor_copy(kpet[:, 128 * s:128 * (s + 1)],
                                      ptp[:, 256:384])

            prev = None
            for s in range(NB):
                if s == 0:
                    xtc = xtc0
                else:
                    xtc = [p12.tile([128, 512], F32R, tag=f"xtc{q}",
                                    name=f"xtc{q}_{s}") for q in range(4)]
                    for q in range(4):
                        nc.sync.dma_start(
                            xtc[q][:], xt.ap()[s][:, 512 * q:512 * (q + 1)])
                pkv = pskv.tile([128, 512], F32, tag="pkv")
                pq1 = psq1.tile([128, 512], F32, tag="pq1")
                pq2 = psq2.tile([128, 320], F32, tag="pq2")
                for k in range(KD):
                    lhs = xtc[k // 4][:, 128 * (k % 4):128 * (k % 4 + 1)]
                    nc.tensor.matmul(pkv[:], lhs, wkvc[k][:],
                                     start=(k == 0),
                                     stop=(k == KD - 1 and not kv_bias),
                                     skip_group_check=True)
                    nc.tensor.matmul(pq1[:], lhs, wqc[k][:, 0:512],
                                     start=(k == 0),
                                     stop=(k == KD - 1 and not q_bias),
                                     skip_group_check=True)
                    nc.tensor.matmul(pq2[:], lhs, wqc[k][:, 512:832],
                                     start=(k == 0),
                                     stop=(k == KD - 1 and not q_bias),
                                     skip_group_check=True)
                if kv_bias:
                    nc.tensor.matmul(pkv[:], ones1[:], kvb_sb[:],
                                     start=False, stop=True,
                                     skip_group_check=True)
                if q_bias:
                    nc.tensor.matmul(pq1[:], ones1[:], qb_sb[:, 0:512],
                                     start=False, stop=True,
                                     skip_group_check=True)
                    nc.tensor.matmul(pq2[:], ones1[:], qb_sb[:, 512:832],
                                     start=False, stop=True,
                                     skip_group_check=True)
                # transposes for the previous block (PE stays ahead)
                if prev is not None:
                    emit_tr(prev)
                # rmsnorm over the 512 latent channels -> kv_bf
                kvtile = p12.tile([128, 512], F32, tag="kvtile")
                nc.scalar.copy(kvtile[:], pkv[:])
                sq = p12.tile([128, 512], F32, tag="sq", bufs=2)
                msq = p12s.tile([128, 1], F32, tag="msq")
                nc.scalar.activation(sq[:], kvtile[:], AF.Square, bias=0.0,
                                     scale=1.0, accum_out=msq[:])
                ms2 = p12s.tile([128, 1], F32, tag="ms2")
                nc.vector.tensor_scalar(ms2[:], msq[:], 1.0 / KV, EPS, ALU.mult,
                                        ALU.add)
                srt = p12s.tile([128, 1], F32, tag="srt")
                nc.scalar.sqrt(srt[:], ms2[:])
                rrt = p12s.tile([128, 1], F32, tag="rrt")
                nc.vector.reciprocal(rrt[:], srt[:])
                kv_bf = p12.tile([128, 512], BF16, tag="kv_bf")
                nc.vector.tensor_scalar(kv_bf[:], kvtile[:], rrt[:], None,
                                        ALU.mult)
                # q nope -> bf16; rope on cols 512:832 (4 q-heads + krope)
                qn_bf = p12.tile([128, 512], BF16, tag="qn_bf")
                nc.scalar.copy(qn_bf[:], pq1[:])
                qr = p12.tile([128, 320], F32, tag="qr")
                nc.scalar.copy(qr[:], pq2[:])
                qsw = p12.tile([128, 320], F32, tag="qsw")
                a3 = qr[:].rearrange("p (i two) -> p i two", two=2)
                w3 = qsw[:].rearrange("p (i two) -> p i two", two=2)
                nc.vector.tensor_copy(w3[:, :, 0:1], a3[:, :, 1:2])
                nc.vector.tensor_copy(w3[:, :, 1:2], a3[:, :, 0:1])
                cs = cosq_sb[:, 64 * s:64 * (s + 1)]
                sn = sinq_sb[:, 64 * s:64 * (s + 1)]
                for r in range(5):
                    rsl = qr[:, 64 * r:64 * (r + 1)]
                    ssl = qsw[:, 64 * r:64 * (r + 1)]
                    nc.vector.tensor_mul(rsl, rsl, cs)
                    nc.vector.tensor_mul(ssl, ssl, sn)
                    nc.vector.tensor_add(rsl, rsl, ssl)
                qro_bf = p12.tile([128, 320], BF16, tag="qro_bf")
                nc.vector.tensor_copy(qro_bf[:], qr[:])
                prev = (s, kv_bf, qn_bf, qro_bf)
            emit_tr(prev)

        # ---------------- persistent tensors (post-P12) ---------------------
        pl = ctx.enter_context(tc.tile_pool(name="pl", bufs=1))
        keff = pl.tile([128, HL * s_len], BF16, tag="keff")   # [d, (h,t)]
        veff = pl.tile([128, HL * s_len], BF16, tag="veff")   # [t, (h,j*d)]
        ot_bf = pl.tile([128, HL * s_len], BF16, tag="ot_bf")  # [d, (h,q)]
        wot_sb = pl.tile([128, HL * D], BF16, tag="wot_sb")
        nc.scalar.dma_start(wot_sb[:], wot.ap()[:])

        # ========== Phase K: materialize per-head K/V =======================
        NG = s_len // 512
        with tc.tile_pool(name="pkef", bufs=2, space="PSUM") as pkef, \
                tc.tile_pool(name="pvef", bufs=3, space="PSUM") as pvef:
            for h in range(HL):
                for tg in range(NG):
                    pk = pkef.tile([128, 512], F32, tag="pk")
                    for cc in range(4):
                        nc.tensor.matmul(
                            pk[:],
                            wbmt_sb[:, 512 * h + 128 * cc:512 * h + 128 * (cc + 1)],
                            kvt[:, s_len * cc + 512 * tg:s_len * cc + 512 * (tg + 1)],
                            start=(cc == 0), stop=(cc == 3))
                    nc.scalar.copy(
                        keff[:, s_len * h + 512 * tg:s_len * h + 512 * (tg + 1)],
                        pk[:])
            for h in range(HL):
                for j in range(NB):
                    pv = pvef.tile([128, 128], F32, tag="pv")
                    for cc in range(4):
                        nc.tensor.matmul(
                            pv[:],
                            kvt[:, s_len * cc + 128 * j:s_len * cc + 128 * (j + 1)],
                            wvt_sb[:, 512 * h + 128 * cc:512 * h + 128 * (cc + 1)],
                            start=(cc == 0), stop=(cc == 3))
                    nc.vector.tensor_copy(
                        veff[:, s_len * h + 128 * j:s_len * h + 128 * (j + 1)],
                        pv[:])

        # ========== Phase 3: attention, software-pipelined over (G, h) ======
        with tc.tile_pool(name="expp", bufs=34) as expp, \
                tc.tile_pool(name="ptp", bufs=3) as ptp, \
                tc.tile_pool(name="zq", bufs=8) as zq, \
                tc.tile_pool(name="pss", bufs=2, space="PSUM") as pss, \
                tc.tile_pool(name="ps3t", bufs=2, space="PSUM") as ps3t, \
                tc.tile_pool(name="ps3o", bufs=2, space="PSUM") as ps3o:

            def emit_scores(h, G):
                """Scores + exp + in-place 1/Z normalize for q-group G."""
                tiles = []
                hb = 64 * (h % 2)
                for ii in range(4):
                    i = 4 * G + ii
                    nk = 128 * (i + 1)
                    nts = (nk + 511) // 512
                    zp = zq.tile([128, 4], F32, tag="zp", bufs=4)
                    row = []
                    for ts in range(nts):
                        t0 = 512 * ts
                        tw = min(512, nk - t0)
                        ps = pss.tile([128, 512], F32, tag="ps")
                        nc.tensor.matmul(
                            ps[:, 0:tw],
                            qnt[:, s_len * h + 128 * i:s_len * h + 128 * (i + 1)],
                            keff[:, s_len * h + t0:s_len * h + t0 + tw],
                            start=True, stop=False, skip_group_check=True)
                        nc.tensor.matmul(
                            ps[:, 0:tw],
                            qpet[h // 2][hb:hb + 64, 128 * i:128 * (i + 1)],
                            kpet[hb:hb + 64, t0:t0 + tw],
                            start=False, stop=True, skip_group_check=True)
                        if t0 + tw == nk:
                            nc.vector.tensor_add(ps[:, tw - 128:tw],
                                                 ps[:, tw - 128:tw], dmask_sb[:])
                        et = expp.tile([128, 512], BF16, tag="expsl", bufs=34)
                        nc.scalar.activation(et[:, 0:tw], ps[:, 0:tw], AF.Exp,
                                             bias=0.0, scale=SCALE,
                                             accum_out=zp[:, ts:ts + 1])
                        row.append(et)
                    if nts > 1:
                        zs = zq.tile([128, 1], F32, tag="zs", bufs=4)
                        nc.vector.reduce_sum(zs[:], zp[:, 0:nts], axis=AX.X)
                    else:
                        zs = zp
                    rq = zq.tile([128, 1], F32, tag="rq", bufs=8)
                    nc.vector.reciprocal(rq[:], zs[:, 0:1])
                    for ts in range(nts):
                        tw = min(512, nk - 512 * ts)
                        nc.gpsimd.tensor_scalar(row[ts][:, 0:tw], row[ts][:, 0:tw],
                                                rq[:], None, ALU.mult)
                    tiles.append(row)
                return tiles

            def emit_strips(h, G, tiles):
                """Transpose normalized P, accumulate PV into oT, store."""
                po = ps3o.tile([128, 512], F32, tag="po")
                for j in range(4 * G + 4):
                    c0 = max(j - 4 * G, 0)
                    pt = ps3t.tile([128, 512], BF16, tag="pt")
                    for ii in range(c0, 4):
                        ts_j, off = divmod(128 * j, 512)
                        nc.tensor.transpose(pt[:, 128 * ii:128 * (ii + 1)],
                                            tiles[ii][ts_j][:, off:off + 128],
                                            identb_sb[:])
                    pts = ptp.tile([128, 512], BF16, tag="pts")
                    nc.vector.tensor_copy(pts[:, 128 * c0:512],
                                          pt[:, 128 * c0:512])
                    # single matmul per strip: PSUM start/stop act on the
                    # whole 2KB zero region, so the bank must have exactly
                    # one start (j==0, full width) and one stop (last strip)
                    nc.tensor.matmul(
                        po[:, 128 * c0:512],
                        veff[:, s_len * h + 128 * j:s_len * h + 128 * (j + 1)],
                        pts[:, 128 * c0:512],
                        start=(j == 0), stop=(j == 4 * G + 3),
                        skip_group_check=True)
                nc.scalar.copy(
                    ot_bf[:, s_len * h + 512 * G:s_len * h + 512 * (G + 1)],
                    po[:])

            work = [(h, G) for G in range(NG) for h in range(HL)]
            pending = None
            for h, G in work:
                tiles = emit_scores(h, G)
                if pending is not None:
                    emit_strips(*pending)
                pending = (h, G, tiles)
            emit_strips(*pending)

        # ========== Phase 4: wo projection (transposed output) ==============
        with tc.tile_pool(name="p4", bufs=3) as p4, \
                tc.tile_pool(name="ps4", bufs=2, space="PSUM") as ps4:
            for G in range(NG):
                for n in range(D // 128):
                    pw = ps4.tile([128, 512], F32, tag="pw")
                    for h in range(HL):
                        nc.tensor.matmul(
                            pw[:],
                            wot_sb[:, D * h + 128 * n:D * h + 128 * (n + 1)],
                            ot_bf[:, s_len * h + 512 * G:s_len * h + 512 * (G + 1)],
                            start=(h == 0), stop=(h == HL - 1))
                    osb = p4.tile([128, 512], F32, tag="osb")
                    nc.scalar.copy(osb[:], pw[:])
                    nc.sync.dma_start(
                        outT.ap()[n][:, 512 * G:512 * (G + 1)], osb[:])

    nc.compile()
    return nc


def make_core_inputs(core, x, freqs, wq_w, wq_b, wkv_a_w, wkv_a_b, kv_norm_w,
                     wkv_b_w, wo_w, s_len):
    """Host-side shard + layout prep for one core."""
    b, g = core // TP, core % TP
    NB = s_len // 128
    heads = [TP * g + hh for hh in range(HL)]  # heads for TP rank g

    ins = {}
    # xt[s, p, 128k+c] = x[b, 128s+c, 128k+p]
    xb = np.ascontiguousarray(x[b, :s_len])                       # [S, D]
    xts = xb.reshape(NB, 128, KD, 128).transpose(0, 3, 2, 1)      # [s, p, k, c]
    ins["xt"] = round_f32r(np.ascontiguousarray(xts).reshape(NB, 128, D))

    # fused q+krope weight: rows = 4x nope(128), 4x qrope(64), krope(64)
    wq3 = wq_w.reshape(H, QK_HD, D)
    rows = [wq3[hg, :NOPE] for hg in heads] + [wq3[hg, NOPE:] for hg in heads]
    rows.append(wkv_a_w[KV:KV + ROPE])                            # krope [64, D]
    wq_sel = np.concatenate(rows, axis=0)                         # [832, D]
    wqt = wq_sel.T.reshape(KD, 128, QW).transpose(1, 0, 2)        # [p, k, 832]
    ins["wq"] = round_f32r(np.ascontiguousarray(wqt).reshape(128, KD * QW))

    wkvt = wkv_a_w[:KV].T.reshape(KD, 128, 512).transpose(1, 0, 2)
    ins["wkv"] = round_f32r(np.ascontiguousarray(wkvt).reshape(128, KD * 512))

    wkv_b3 = wkv_b_w.reshape(H, NOPE + V_HD, KV)
    # wbmt: per head, (wb_h * norm)^T in 4 chunks of [128c, 128d]
    wbt_cols = []
    for hg in heads:
        wb = wkv_b3[hg, :NOPE] * kv_norm_w[None, :]               # [128d, 512c]
        wbt_cols.append(wb.T.reshape(4, 128, 128).transpose(1, 0, 2).reshape(128, 512))
    ins["wbmt"] = to_bf16(np.concatenate(wbt_cols, axis=1))       # [128, 4*512]

    wvt_cols = []
    for hg in heads:
        wv = wkv_b3[hg, NOPE:] * kv_norm_w[None, :]               # [128d, 512c]
        wvt_cols.append(wv.T.reshape(4, 128, 128).transpose(1, 0, 2).reshape(128, 512))
    ins["wvt"] = to_bf16(np.concatenate(wvt_cols, axis=1))        # [128, 4*512]

    wo_cols = np.concatenate([wo_w[:, hg * V_HD:(hg + 1) * V_HD] for hg in heads],
                             axis=1)                              # [D, 512]
    wotl = wo_cols.T.reshape(HL, 128, D).transpose(1, 0, 2)       # [d, h, D]
    ins["wot"] = to_bf16(wotl.reshape(128, HL * D))

    # rope tables in [s-block(128), 64] free-pair layout
    fr = freqs[:s_len]                                            # [S, 32]
    cos2 = np.repeat(np.cos(fr), 2, axis=1).astype(np.float32)    # [S, 64]
    sin1 = np.sin(fr)
    sin2 = np.empty((s_len, ROPE), np.float32)
    sin2[:, 0::2] = -sin1
    sin2[:, 1::2] = sin1
    ins["cosq"] = np.ascontiguousarray(
        cos2.reshape(NB, 128, 64).transpose(1, 0, 2).reshape(128, NB * 64))
    ins["sinq"] = np.ascontiguousarray(
        sin2.reshape(NB, 128, 64).transpose(1, 0, 2).reshape(128, NB * 64))

    ins["dmask"] = np.where(np.triu(np.ones((128, 128), bool), k=1),
                            np.float32(NEG), np.float32(0.0))
    ins["identb"] = to_bf16(np.eye(128, dtype=np.float32))

    if np.any(wq_b != 0.0):
        rows_b = [wq_b.reshape(H, QK_HD)[hg, :NOPE] for hg in heads] + \
                 [wq_b.reshape(H, QK_HD)[hg, NOPE:] for hg in heads]
        rows_b.append(wkv_a_b[KV:KV + ROPE])
        ins["qb"] = round_f32r(np.concatenate(rows_b)[None, :])
    if np.any(wkv_a_b != 0.0):
        ins["kvb"] = round_f32r(wkv_a_b[:KV][None, :])
    return ins


_nc_cache = {}


def get_nc(s_len, q_bias, kv_bias):
    key = (s_len, q_bias, kv_bias)
    if key not in _nc_cache:
        _nc_cache[key] = build(s_len, q_bias, kv_bias)
    return _nc_cache[key]


def run_cores(inputs, s_len=S, trace=False):
    """Build per-core shards, run the SPMD kernel, return (out, results)."""
    x = np.asarray(inputs["x"], np.float32)
    freqs = np.asarray(inputs["freqs"], np.float32)
    wq_w = np.asarray(inputs["wq_w"], np.float32)
    wq_b = np.asarray(inputs["wq_b"], np.float32)
    wkv_a_w = np.asarray(inputs["wkv_a_w"], np.float32)
    wkv_a_b = np.asarray(inputs["wkv_a_b"], np.float32)
    kv_norm_w = np.asarray(inputs["kv_norm_w"], np.float32)
    wkv_b_w = np.asarray(inputs["wkv_b_w"], np.float32)
    wo_w = np.asarray(inputs["wo_w"], np.float32)
    wo_b = np.asarray(inputs["wo_b"], np.float32)

    q_bias = bool(np.any(wq_b != 0.0) or np.any(wkv_a_b[KV:] != 0.0))
    kv_bias = bool(np.any(wkv_a_b[:KV] != 0.0))
    nc = get_nc(s_len, q_bias, kv_bias)
    in_maps = [
        make_core_inputs(c, x, freqs, wq_w, wq_b, wkv_a_w, wkv_a_b, kv_norm_w,
                         wkv_b_w, wo_w, s_len)
        for c in range(N_CORES)
    ]
    res = bass_utils.run_bass_kernel_spmd(nc, in_maps, core_ids=list(range(N_CORES)),
                                          trace=trace)
    out = np.empty((B, s_len, D), np.float32)
    for b in range(B):
        p = [np.asarray(res.results[TP * b + g]["outT"], np.float32)
                .reshape(D, s_len).T
             for g in range(TP)]
        out[b] = (p[0] + p[1]) + (p[2] + p[3])
    out += wo_b[None, None, :]
    return out, res


def kernel(**inputs) -> np.ndarray:
    out, _ = run_cores(inputs, s_len=S, trace=False)
    return out


# revision 20
# speedup vs baseline: 1.7049x; 1.0089x over previous
"""MLA (DeepSeek-style multi-head latent attention) kernel for Trainium2, v2.

Problem: nn_MultiHeadAttention_28243704939173
  B=2, S=2048, D=2048, H=16, KV_RANK=512, NOPE=128, ROPE=64, V_HD=128.

Sharding (8 NeuronCores): DP=2 over batch x TP=4 over heads (4 heads per
core); the kv latent is computed replicated on every TP rank. Each core
produces its heads' partial wo projection (transposed, [D, S]); the host
sums the 4 TP partials per batch element and adds wo_b.

v2 structure (vs v1):
  * Per-head K/V materialization: k_eff_h = wb_h @ kv_latent^T (128-d) and
    v_eff_h = kv_latent @ wv_h^T (128-d) are computed once (cheap GEMMs),
    so scores contract over 192 dims and PV over per-128 blocks instead of
    doing both in the 512-d latent space. ~2.4x less PE work in attention.
  * Attention path entirely in bf16 (inputs quantized ~0.1%; final rel err
    ~1e-3 vs the 2e-2 gate). x-projections stay fp32r.
  * Single fused pass over x computes kv latent + q (nope/rope) + k-rope
    in one GEMM sweep (x loaded once).
  * All weights prefetched on the Activation DMA queue while x streams on
    the SP queue; no DRAM bounces for intermediates.
  * Softmax normalization folded into P tiles in-place (gpsimd) before the
    PE transposes; per-(head, q-group) work software-pipelined so the PE
    never waits on the softmax chain.
"""
import numpy as np
from contextlib import ExitStack

import ml_dtypes

import concourse.bass as bass
import concourse.bacc as bacc
import concourse.mybir as mybir
import concourse.tile as tile
from concourse import bass_utils

F32 = mybir.dt.float32
F32R = mybir.dt.float32r
BF16 = mybir.dt.bfloat16
AF = mybir.ActivationFunctionType
ALU = mybir.AluOpType
AX = mybir.AxisListType

B, S, D = 2, 2048, 2048
H = 16
KV = 512
NOPE, ROPE = 128, 64
QK_HD = NOPE + ROPE
V_HD = 128
SCALE = float(QK_HD) ** -0.5
EPS = 1.1920929e-07
NEG = -1.0e5  # mask addend; NEG*SCALE ~ -7220 -> exp underflows to exactly 0
HL = 4        # local heads per core (TP degree 4)
TP = 4
N_CORES = 8
KD = D // 128   # contraction chunks over the model dim
QW = 832        # fused q-projection width: 4*128 nope + 4*64 qrope + 64 krope


def round_f32r(a: np.ndarray) -> np.ndarray:
    """Round fp32 -> fp32r (11-bit mantissa, RNE), keeping fp32 container."""
    u = np.ascontiguousarray(a, dtype=np.float32).view(np.uint32).copy()
    lsb = (u >> np.uint32(12)) & np.uint32(1)
    u += np.uint32(0x7FF) + lsb
    u &= np.uint32(0xFFFFF000)
    return u.view(np.float32)


def to_bf16(a: np.ndarray) -> np.ndarray:
    return np.ascontiguousarray(a).astype(ml_dtypes.bfloat16)


def build(s_len: int, q_bias: bool, kv_bias: bool):
    NB = s_len // 128

    nc = bacc.Bacc("TRN2", target_bir_lowering=False, debug=False)

    xt = nc.dram_tensor("xt", [NB, 128, D], F32R, kind="ExternalInput")
    wq = nc.dram_tensor("wq", [128, KD * QW], F32R, kind="ExternalInput")
    wkv = nc.dram_tensor("wkv", [128, KD * 512], F32R, kind="ExternalInput")
    wbmt = nc.dram_tensor("wbmt", [128, HL * 512], BF16, kind="ExternalInput")
    wvt = nc.dram_tensor("wvt", [128, HL * 512], BF16, kind="ExternalInput")
    wot = nc.dram_tensor("wot", [128, HL * D], BF16, kind="ExternalInput")
    cosq = nc.dram_tensor("cosq", [128, NB * 64], F32, kind="ExternalInput")
    sinq = nc.dram_tensor("sinq", [128, NB * 64], F32, kind="ExternalInput")
    dmask = nc.dram_tensor("dmask", [128, 128], F32, kind="ExternalInput")
    identb = nc.dram_tensor("identb", [128, 128], BF16, kind="ExternalInput")
    if q_bias:
        qb = nc.dram_tensor("qb", [1, QW], F32R, kind="ExternalInput")
    if kv_bias:
        kvb = nc.dram_tensor("kvb", [1, 512], F32R, kind="ExternalInput")
    outT = nc.dram_tensor("outT", [128, D // 128, s_len], F32,
                          kind="ExternalOutput")

    with tile.TileContext(nc) as tc, ExitStack() as ctx:
        # ---------------- persistent tensors (whole kernel) -----------------
        pe = ctx.enter_context(tc.tile_pool(name="pe", bufs=1))
        identb_sb = pe.tile([128, 128], BF16, tag="identb_sb")
        dmask_sb = pe.tile([128, 128], F32, tag="dmask_sb")
        qnt = pe.tile([128, HL * s_len], BF16, tag="qnt")       # [d, (h,q)]
        qpet = [pe.tile([128, s_len], BF16, tag=f"qpet{pp}", name=f"qpet{pp}")
                for pp in range(2)]                             # [(2h,r), q]
        kpet = pe.tile([128, s_len], BF16, tag="kpet")          # [r x2, t]
        kvt = pe.tile([128, 4 * s_len], BF16, tag="kvt")        # [c, (cc,t)]
        wbmt_sb = pe.tile([128, HL * 512], BF16, tag="wbmt_sb")
        wvt_sb = pe.tile([128, HL * 512], BF16, tag="wvt_sb")

        nc.sync.dma_start(identb_sb[:], identb.ap()[:])
        nc.sync.dma_start(dmask_sb[:], dmask.ap()[:])

        # ========== Phase 12: fused kv-latent + q/k-rope projections ========
        with tc.tile_pool(name="p12w", bufs=1) as p12w, \
                tc.tile_pool(name="p12", bufs=3) as p12, \
                tc.tile_pool(name="p12s", bufs=4) as p12s, \
                tc.tile_pool(name="pskv", bufs=2, space="PSUM") as pskv, \
                tc.tile_pool(name="psq1", bufs=2, space="PSUM") as psq1, \
                tc.tile_pool(name="psq2", bufs=2, space="PSUM") as psq2, \
                tc.tile_pool(name="pst", bufs=2, space="PSUM") as pst:
            # x block 0 first on the SP queue so compute can start early
            xtc0 = [p12.tile([128, 512], F32R, tag=f"xtc{q}", name=f"xtc{q}_0")
                    for q in range(4)]
            for q in range(4):
                nc.sync.dma_start(xtc0[q][:], xt.ap()[0][:, 512 * q:512 * (q + 1)])
            # kv weights stream on the SP queue behind x block 0; q weights
            # on the Activation HWDGE queue in parallel
            wkvc = []
            for k in range(KD):
                wt = p12w.tile([128, 512], F32R, tag="wkvc", name=f"wkvc{k}",
                               bufs=KD)
                nc.sync.dma_start(wt[:], wkv.ap()[:, 512 * k:512 * (k + 1)])
                wkvc.append(wt)
            cosq_sb = p12w.tile([128, NB * 64], F32, tag="cosq_sb")
            sinq_sb = p12w.tile([128, NB * 64], F32, tag="sinq_sb")
            wqc = []
            for k in range(KD):
                wt = p12w.tile([128, QW], F32R, tag="wqc", name=f"wqc{k}",
                               bufs=KD)
                nc.scalar.dma_start(wt[:], wq.ap()[:, QW * k:QW * (k + 1)])
                wqc.append(wt)
                if k == 3:
                    nc.scalar.dma_start(cosq_sb[:], cosq.ap()[:])
                    nc.scalar.dma_start(sinq_sb[:], sinq.ap()[:])
            nc.scalar.dma_start(wbmt_sb[:], wbmt.ap()[:])
            nc.scalar.dma_start(wvt_sb[:], wvt.ap()[:])
            if q_bias or kv_bias:
                ones1 = p12w.tile([1, 128], F32R, tag="ones1")
                nc.vector.memset(ones1[:], 1.0)
            if q_bias:
                qb_sb = p12w.tile([1, QW], F32R, tag="qb_sb")
                nc.scalar.dma_start(qb_sb[:], qb.ap()[:])
            if kv_bias:
                kvb_sb = p12w.tile([1, 512], F32R, tag="kvb_sb")
                nc.scalar.dma_start(kvb_sb[:], kvb.ap()[:])

            qnt4 = qnt[:].rearrange("p (h n c) -> p h n c", h=HL, n=NB)
            kvt4 = kvt[:].rearrange("p (cc n c) -> p cc n c", cc=4, n=NB)

            def emit_tr(st):
                """PE transposes for block st (lagged one iteration)."""
                s, kv_bf, qn_bf, qro_bf = st
                ptc = pst.tile([128, 512], BF16, tag="ptc")
                for cc in range(4):
                    nc.tensor.transpose(ptc[:, 128 * cc:128 * (cc + 1)],
                                        kv_bf[:, 128 * cc:128 * (cc + 1)],
                                        identb_sb[:])
                nc.vector.tensor_copy(
                    kvt4[:, :, s, :],
                    ptc[:].rearrange("p (cc c) -> p cc c", cc=4))
                ptq = pst.tile([128, 512], BF16, tag="ptc")
                for hh in range(HL):
                    nc.tensor.transpose(ptq[:, 128 * hh:128 * (hh + 1)],
                                        qn_bf[:, 128 * hh:128 * (hh + 1)],
                                        identb_sb[:])
                nc.vector.tensor_copy(
                    qnt4[:, :, s, :],
                    ptq[:].rearrange("p (h c) -> p h c", h=HL))
                ptp = pst.tile([128, 512], BF16, tag="ptc")
                for pp in range(2):
                    nc.tensor.transpose(ptp[:, 128 * pp:128 * (pp + 1)],
                                        qro_bf[:, 128 * pp:128 * (pp + 1)],
                                        identb_sb[:])
                    nc.vector.tensor_copy(qpet[pp][:, 128 * s:128 * (s + 1)],
                                          ptp[:, 128 * pp:128 * (pp + 1)])
                # kpeT duplicated on both partition halves (the rope matmul
                # pairs it with either half of qpet); DVE cannot cross
                # partitions, so transpose twice with an explicit placement.
                nc.tensor.transpose(ptp[0:64, 256:384], qro_bf[:, 256:320],
                                    identb_sb[:], tile_position=(0, 0))
                nc.tensor.transpose(ptp[64:128, 256:384], qro_bf[:, 256:320],
                                    identb_sb[:], tile_position=(0, 64))
                nc.vector.tensor_copy(kpet[:, 128 * s:128 * (s + 1)],
                                      ptp[:, 256:384])

            prev = None
            for s in range(NB):
                if s == 0:
                    xtc = xtc0
                else:
                    xtc = [p12.tile([128, 512], F32R, tag=f"xtc{q}",
                                    name=f"xtc{q}_{s}") for q in range(4)]
                    for q in range(4):
                        nc.sync.dma_start(
                            xtc[q][:], xt.ap()[s][:, 512 * q:512 * (q + 1)])
                pkv = pskv.tile([128, 512], F32, tag="pkv")
                pq1 = psq1.tile([128, 512], F32, tag="pq1")
                pq2 = psq2.tile([128, 320], F32, tag="pq2")
                for k in range(KD):
                    lhs = xtc[k // 4][:, 128 * (k % 4):128 * (k % 4 + 1)]
                    nc.tensor.matmul(pkv[:], lhs, wkvc[k][:],
                                     start=(k == 0),
                                     stop=(k == KD - 1 and not kv_bias),
                                     skip_group_check=True)
                    nc.tensor.matmul(pq1[:], lhs, wqc[k][:, 0:512],
                                     start=(k == 0),
                                     stop=(k == KD - 1 and not q_bias),
                                     skip_group_check=True)
                    nc.tensor.matmul(pq2[:], lhs, wqc[k][:, 512:832],
                                     start=(k == 0),
                                     stop=(k == KD - 1 and not q_bias),
                                     skip_group_check=True)
                if kv_bias:
                    nc.tensor.matmul(pkv[:], ones1[:], kvb_sb[:],
                                     start=False, stop=True,
                                     skip_group_check=True)
                if q_bias:
                    nc.tensor.matmul(pq1[:], ones1[:], qb_sb[:, 0:512],
                                     start=False, stop=True,
                                     skip_group_check=True)
                    nc.tensor.matmul(pq2[:], ones1[:], qb_sb[:, 512:832],
                                     start=False, stop=True,
                                     skip_group_check=True)
                # transposes for the previous block (PE stays ahead)
                if prev is not None:
                    emit_tr(prev)
                # rmsnorm over the 512 latent channels -> kv_bf
                kvtile = p12.tile([128, 512], F32, tag="kvtile")
                nc.scalar.copy(kvtile[:], pkv[:])
                sq = p12.tile([128, 512], F32, tag="sq", bufs=2)
                msq = p12s.tile([128, 1], F32, tag="msq")
                nc.scalar.activation(sq[:], kvtile[:], AF.Square, bias=0.0,
                                     scale=1.0, accum_out=msq[:])
                ms2 = p12s.tile([128, 1], F32, tag="ms2")
                nc.vector.tensor_scalar(ms2[:], msq[:], 1.0 / KV, EPS, ALU.mult,
                                        ALU.add)
                srt = p12s.tile([128, 1], F32, tag="srt")
                nc.scalar.sqrt(srt[:], ms2[:])
                rrt = p12s.tile([128, 1], F32, tag="rrt")
                nc.vector.reciprocal(rrt[:], srt[:])
                kv_bf = p12.tile([128, 512], BF16, tag="kv_bf")
                nc.vector.tensor_scalar(kv_bf[:], kvtile[:], rrt[:], None,
                                        ALU.mult)
                # q nope -> bf16; rope on cols 512:832 (4 q-heads + krope)
                qn_bf = p12.tile([128, 512], BF16, tag="qn_bf")
                nc.scalar.copy(qn_bf[:], pq1[:])
                qr = p12.tile([128, 320], F32, tag="qr")
                nc.scalar.copy(qr[:], pq2[:])
                qsw = p12.tile([128, 320], F32, tag="qsw")
                a3 = qr[:].rearrange("p (i two) -> p i two", two=2)
                w3 = qsw[:].rearrange("p (i two) -> p i two", two=2)
                nc.vector.tensor_copy(w3[:, :, 0:1], a3[:, :, 1:2])
                nc.vector.tensor_copy(w3[:, :, 1:2], a3[:, :, 0:1])
                cs = cosq_sb[:, 64 * s:64 * (s + 1)]
                sn = sinq_sb[:, 64 * s:64 * (s + 1)]
                for r in range(5):
                    rsl = qr[:, 64 * r:64 * (r + 1)]
                    ssl = qsw[:, 64 * r:64 * (r + 1)]
                    nc.vector.tensor_mul(rsl, rsl, cs)
                    nc.vector.tensor_mul(ssl, ssl, sn)
                    nc.vector.tensor_add(rsl, rsl, ssl)
                qro_bf = p12.tile([128, 320], BF16, tag="qro_bf")
                nc.vector.tensor_copy(qro_bf[:], qr[:])
                prev = (s, kv_bf, qn_bf, qro_bf)
            emit_tr(prev)

        # ---------------- persistent tensors (post-P12) ---------------------
        pl = ctx.enter_context(tc.tile_pool(name="pl", bufs=1))
        keff = pl.tile([128, HL * s_len], BF16, tag="keff")   # [d, (h,t)]
        veff = pl.tile([128, HL * s_len], BF16, tag="veff")   # [t, (h,j*d)]
        ot_bf = pl.tile([128, HL * s_len], BF16, tag="ot_bf")  # [d, (h,q)]
        wot_sb = pl.tile([128, HL * D], BF16, tag="wot_sb")
        nc.scalar.dma_start(wot_sb[:], wot.ap()[:])

        # ========== Phase K: materialize per-head K/V =======================
        NG = s_len // 512
        with tc.tile_pool(name="pkef", bufs=2, space="PSUM") as pkef, \
                tc.tile_pool(name="pvef", bufs=3, space="PSUM") as pvef:
            for h in range(HL):
                for tg in range(NG):
                    pk = pkef.tile([128, 512], F32, tag="pk")
                    for cc in range(4):
                        nc.tensor.matmul(
                            pk[:],
                            wbmt_sb[:, 512 * h + 128 * cc:512 * h + 128 * (cc + 1)],
                            kvt[:, s_len * cc + 512 * tg:s_len * cc + 512 * (tg + 1)],
                            start=(cc == 0), stop=(cc == 3))
                    nc.scalar.copy(
                        keff[:, s_len * h + 512 * tg:s_len * h + 512 * (tg + 1)],
                        pk[:])
            for h in range(HL):
                for jg in range(NB // 4):
                    pv = pvef.tile([128, 512], F32, tag="pv")
                    for jj in range(4):
                        j = 4 * jg + jj
                        for cc in range(4):
                            nc.tensor.matmul(
                                pv[:, 128 * jj:128 * (jj + 1)],
                                kvt[:, s_len * cc + 128 * j:s_len * cc + 128 * (j + 1)],
                                wvt_sb[:, 512 * h + 128 * cc:512 * h + 128 * (cc + 1)],
                                start=(cc == 0), stop=(cc == 3),
                                skip_group_check=True)
                    nc.vector.tensor_copy(
                        veff[:, s_len * h + 512 * jg:s_len * h + 512 * (jg + 1)],
                        pv[:])

        # ========== Phase 3: attention, software-pipelined over (G, h) ======
        with tc.tile_pool(name="expp", bufs=34) as expp, \
                tc.tile_pool(name="ptp", bufs=3) as ptp, \
                tc.tile_pool(name="dgp", bufs=8) as dgp, \
                tc.tile_pool(name="zq", bufs=8) as zq, \
                tc.tile_pool(name="pss", bufs=3, space="PSUM") as pss, \
                tc.tile_pool(name="ps3t", bufs=2, space="PSUM") as ps3t, \
                tc.tile_pool(name="ps3o", bufs=2, space="PSUM") as ps3o:

            def emit_scores(h, G):
                """Scores + exp + diag(1/Z) per q-block."""
                tiles = []
                diags = []
                hb = 64 * (h % 2)
                for ii in range(4):
                    i = 4 * G + ii
                    nk = 128 * (i + 1)
                    nts = (nk + 511) // 512
                    zp = zq.tile([128, 4], F32, tag="zp", bufs=4)
                    row = []
                    for ts in range(nts):
                        t0 = 512 * ts
                        tw = min(512, nk - t0)
                        ps = pss.tile([128, 512], F32, tag="ps")
                        nc.tensor.matmul(
                            ps[:, 0:tw],
                            qnt[:, s_len * h + 128 * i:s_len * h + 128 * (i + 1)],
                            keff[:, s_len * h + t0:s_len * h + t0 + tw],
                            start=True, stop=False, skip_group_check=True)
                        nc.tensor.matmul(
                            ps[:, 0:tw],
                            qpet[h // 2][hb:hb + 64, 128 * i:128 * (i + 1)],
                            kpet[hb:hb + 64, t0:t0 + tw],
                            start=False, stop=True, skip_group_check=True)
                        if t0 + tw == nk:
                            nc.vector.tensor_add(ps[:, tw - 128:tw],
                                                 ps[:, tw - 128:tw], dmask_sb[:])
                        et = expp.tile([128, 512], BF16, tag="expsl", bufs=34)
                        nc.scalar.activation(et[:, 0:tw], ps[:, 0:tw], AF.Exp,
                                             bias=0.0, scale=SCALE,
                                             accum_out=zp[:, ts:ts + 1])
                        row.append(et)
                    if nts > 1:
                        zs = zq.tile([128, 1], F32, tag="zs", bufs=4)
                        nc.vector.reduce_sum(zs[:], zp[:, 0:nts], axis=AX.X)
                    else:
                        zs = zp
                    rq = zq.tile([128, 1], F32, tag="rq", bufs=8)
                    nc.vector.reciprocal(rq[:], zs[:, 0:1])
                    dg = dgp.tile([128, 128], BF16, tag="dg", bufs=8)
                    nc.gpsimd.tensor_scalar(dg[:], identb_sb[:], rq[:], None,
                                            ALU.mult)
                    tiles.append(row)
                    diags.append(dg)
                return tiles, diags

            def emit_strips(h, G, tiles, diags):
                """P^T * diag(1/Z) via transpose, accumulate PV into oT."""
                po = ps3o.tile([128, 512], F32, tag="po")
                for j in range(4 * G + 4):
                    c0 = max(j - 4 * G, 0)
                    pt = ps3t.tile([128, 512], F32, tag="pt")
                    for ii in range(c0, 4):
                        ts_j, off = divmod(128 * j, 512)
                        # out[t, q] = sum_q' exp[q', t] * diag[q', q]
                        #           = P^T normalized by 1/Z in the same pass
                        nc.tensor.matmul(pt[:, 128 * ii:128 * (ii + 1)],
                                         tiles[ii][ts_j][:, off:off + 128],
                                         diags[ii][:],
                                         start=True, stop=True,
                                         skip_group_check=True)
                    pts = ptp.tile([128, 512], BF16, tag="pts")
                    if j % 2 == 0:
                        nc.vector.tensor_copy(pts[:, 128 * c0:512],
                                              pt[:, 128 * c0:512])
                    else:
                        nc.scalar.copy(pts[:, 128 * c0:512],
                                       pt[:, 128 * c0:512])
                    # single matmul per strip: PSUM start/stop act on the
                    # whole 2KB zero region, so the bank must have exactly
                    # one start (j==0, full width) and one stop (last strip)
                    nc.tensor.matmul(
                        po[:, 128 * c0:512],
                        veff[:, s_len * h + 128 * j:s_len * h + 128 * (j + 1)],
                        pts[:, 128 * c0:512],
                        start=(j == 0), stop=(j == 4 * G + 3),
                        skip_group_check=True)
                odst = ot_bf[:, s_len * h + 512 * G:s_len * h + 512 * (G + 1)]
                if (h + G) % 2 == 0:
                    nc.scalar.copy(odst, po[:])
                else:
                    nc.vector.tensor_copy(odst, po[:])

            work = [(h, G) for G in range(NG) for h in range(HL)]
            pending = None
            for h, G in work:
                tiles, diags = emit_scores(h, G)
                if pending is not None:
                    emit_strips(*pending)
                pending = (h, G, tiles, diags)
            emit_strips(*pending)

        # ========== Phase 4: wo projection (transposed output) ==============
        with tc.tile_pool(name="p4", bufs=3) as p4, \
                tc.tile_pool(name="ps4", bufs=2, space="PSUM") as ps4:
            for G in range(NG):
                for ng in range(D // 512):
                    osb = p4.tile([128, 2048], F32, tag="osb")
                    for nn in range(4):
                        n = 4 * ng + nn
                        pw = ps4.tile([128, 512], F32, tag="pw")
                        for h in range(HL):
                            nc.tensor.matmul(
                                pw[:],
                                wot_sb[:, D * h + 128 * n:D * h + 128 * (n + 1)],
                                ot_bf[:, s_len * h + 512 * G:s_len * h + 512 * (G + 1)],
                                start=(h == 0), stop=(h == HL - 1))
                        nc.scalar.copy(osb[:, 512 * nn:512 * (nn + 1)], pw[:])
                    nc.sync.dma_start(
                        outT.ap()[:, 4 * ng:4 * (ng + 1), 512 * G:512 * (G + 1)],
                        osb[:].rearrange("p (n q) -> p n q", n=4))

    nc.compile()
    return nc


def make_core_inputs(core, x, freqs, wq_w, wq_b, wkv_a_w, wkv_a_b, kv_norm_w,
                     wkv_b_w, wo_w, s_len):
    """Host-side shard + layout prep for one core."""
    b, g = core // TP, core % TP
    NB = s_len // 128
    heads = [TP * g + hh for hh in range(HL)]  # heads for TP rank g

    ins = {}
    # xt[s, p, 128k+c] = x[b, 128s+c, 128k+p]
    xb = np.ascontiguousarray(x[b, :s_len])                       # [S, D]
    xts = xb.reshape(NB, 128, KD, 128).transpose(0, 3, 2, 1)      # [s, p, k, c]
    ins["xt"] = round_f32r(np.ascontiguousarray(xts).reshape(NB, 128, D))

    # fused q+krope weight: rows = 4x nope(128), 4x qrope(64), krope(64)
    wq3 = wq_w.reshape(H, QK_HD, D)
    rows = [wq3[hg, :NOPE] for hg in heads] + [wq3[hg, NOPE:] for hg in heads]
    rows.append(wkv_a_w[KV:KV + ROPE])                            # krope [64, D]
    wq_sel = np.concatenate(rows, axis=0)                         # [832, D]
    wqt = wq_sel.T.reshape(KD, 128, QW).transpose(1, 0, 2)        # [p, k, 832]
    ins["wq"] = round_f32r(np.ascontiguousarray(wqt).reshape(128, KD * QW))

    wkvt = wkv_a_w[:KV].T.reshape(KD, 128, 512).transpose(1, 0, 2)
    ins["wkv"] = round_f32r(np.ascontiguousarray(wkvt).reshape(128, KD * 512))

    wkv_b3 = wkv_b_w.reshape(H, NOPE + V_HD, KV)
    # wbmt: per head, (wb_h * norm)^T in 4 chunks of [128c, 128d]
    wbt_cols = []
    for hg in heads:
        wb = wkv_b3[hg, :NOPE] * kv_norm_w[None, :]               # [128d, 512c]
        wbt_cols.append(wb.T.reshape(4, 128, 128).transpose(1, 0, 2).reshape(128, 512))
    ins["wbmt"] = to_bf16(np.concatenate(wbt_cols, axis=1))       # [128, 4*512]

    wvt_cols = []
    for hg in heads:
        wv = wkv_b3[hg, NOPE:] * kv_norm_w[None, :]               # [128d, 512c]
        wvt_cols.append(wv.T.reshape(4, 128, 128).transpose(1, 0, 2).reshape(128, 512))
    ins["wvt"] = to_bf16(np.concatenate(wvt_cols, axis=1))        # [128, 4*512]

    wo_cols = np.concatenate([wo_w[:, hg * V_HD:(hg + 1) * V_HD] for hg in heads],
                             axis=1)                              # [D, 512]
    wotl = wo_cols.T.reshape(HL, 128, D).transpose(1, 0, 2)       # [d, h, D]
    ins["wot"] = to_bf16(wotl.reshape(128, HL * D))

    # rope tables in [s-block(128), 64] free-pair layout
    fr = freqs[:s_len]                                            # [S, 32]
    cos2 = np.repeat(np.cos(fr), 2, axis=1).astype(np.float32)    # [S, 64]
    sin1 = np.sin(fr)
    sin2 = np.empty((s_len, ROPE), np.float32)
    sin2[:, 0::2] = -sin1
    sin2[:, 1::2] = sin1
    ins["cosq"] = np.ascontiguousarray(
        cos2.reshape(NB, 128, 64).transpose(1, 0, 2).reshape(128, NB * 64))
    ins["sinq"] = np.ascontiguousarray(
        sin2.reshape(NB, 128, 64).transpose(1, 0, 2).reshape(128, NB * 64))

    ins["dmask"] = np.where(np.triu(np.ones((128, 128), bool), k=1),
                            np.float32(NEG), np.float32(0.0))
    ins["identb"] = to_bf16(np.eye(128, dtype=np.float32))

    if np.any(wq_b != 0.0):
        rows_b = [wq_b.reshape(H, QK_HD)[hg, :NOPE] for hg in heads] + \
                 [wq_b.reshape(H, QK_HD)[hg, NOPE:] for hg in heads]
        rows_b.append(wkv_a_b[KV:KV + ROPE])
        ins["qb"] = round_f32r(np.concatenate(rows_b)[None, :])
    if np.any(wkv_a_b != 0.0):
        ins["kvb"] = round_f32r(wkv_a_b[:KV][None, :])
    return ins


_nc_cache = {}


def get_nc(s_len, q_bias, kv_bias):
    key = (s_len, q_bias, kv_bias)
    if key not in _nc_cache:
        _nc_cache[key] = build(s_len, q_bias, kv_bias)
    return _nc_cache[key]


def run_cores(inputs, s_len=S, trace=False):
    """Build per-core shards, run the SPMD kernel, return (out, results)."""
    x = np.asarray(inputs["x"], np.float32)
    freqs = np.asarray(inputs["freqs"], np.float32)
    wq_w = np.asarray(inputs["wq_w"], np.float32)
    wq_b = np.asarray(inputs["wq_b"], np.float32)
    wkv_a_w = np.asarray(inputs["wkv_a_w"], np.float32)
    wkv_a_b = np.asarray(inputs["wkv_a_b"], np.float32)
    kv_norm_w = np.asarray(inputs["kv_norm_w"], np.float32)
    wkv_b_w = np.asarray(inputs["wkv_b_w"], np.float32)
    wo_w = np.asarray(inputs["wo_w"], np.float32)
    wo_b = np.asarray(inputs["wo_b"], np.float32)

    q_bias = bool(np.any(wq_b != 0.0) or np.any(wkv_a_b[KV:] != 0.0))
    kv_bias = bool(np.any(wkv_a_b[:KV] != 0.0))
    nc = get_nc(s_len, q_bias, kv_bias)
    in_maps = [
        make_core_inputs(c, x, freqs, wq_w, wq_b, wkv_a_w, wkv_a_b, kv_norm_w,
                         wkv_b_w, wo_w, s_len)
        for c in range(N_CORES)
    ]
    res = bass_utils.run_bass_kernel_spmd(nc, in_maps, core_ids=list(range(N_CORES)),
                                          trace=trace)
    out = np.empty((B, s_len, D), np.float32)
    for b in range(B):
        p = [np.asarray(res.results[TP * b + g]["outT"], np.float32)
                .transpose(1, 0, 2).reshape(D, s_len).T
             for g in range(TP)]
        out[b] = (p[0] + p[1]) + (p[2] + p[3])
    out += wo_b[None, None, :]
    return out, res


def kernel(**inputs) -> np.ndarray:
    out, _ = run_cores(inputs, s_len=S, trace=False)
    return out
